# revision 2
# baseline (speedup 1.0000x reference)
"""Self-attention kernel for Trainium2 (8 NeuronCores, SPMD).

Problem: X[8192,512], Wq,Wk[512,512]:
    Q = X@Wq ; K = X@Wk ; S = softmax(Q K^T / sqrt(512)) ; out = S @ X

Sharding: rows of Q (query blocks of 1024) across 8 cores; K/V (=X) replicated.

Per-core dataflow (core owns query rows i in [c*1024, (c+1)*1024)):
  Phase P:  Q^T = (Wq^T X_mine^T)/sqrt(d)   [512,1024]   (resident, f32r)
            K^T = Wk^T X^T                  [512,8192]   (spilled to DRAM, f32r)
  Per i-half h (512 query columns):
    B1: for each j-tile (64): S^T tile [j=128, i=512] = sum_d K^T_tile.T Q^T
        (4 accumulating f32r matmuls) -> ACT copy PSUM->SBUF (S region),
        DVE running elementwise max -> mx[128,512]
    B2: partition-reduce mx via PE transpose + DVE reduce_max -> m[1,512];
        broadcast back to [128,512] via ones outer-product matmul
    B3: for each j-tile: d = S^T_t - B (DVE), clamp(-80) (DVE), exp (ACT,
        f32r out) -> P~; PE: 4 accumulating matmuls o[i-chunk,512v] +=
        P~[:,chunk].T @ X[j-tile] and 1 ones-matmul sum[1,512] += col-sums
    B4: recip(sum), transpose to per-partition cols, ACT Copy-with-scale
        drains o PSUM -> SBUF, DMA out.

The host supplies X, X^T and the per-core X^T slice as separate inputs
(layout staging only; all FLOPs happen on device). fp32r matmuls keep
~13 mantissa bits => logit noise ~0.08 => output rel err ~3e-3.
"""
import sys

sys.path.insert(0, "/opt/trn_rl_repo")

import numpy as np

import concourse.bass as bass
import concourse.mybir as mybir
import concourse.tile as tile
from concourse import bacc
from concourse.bass import ts
from concourse.bass_utils import run_bass_kernel_spmd
from concourse.masks import make_identity

F32 = mybir.dt.float32
F32R = mybir.dt.float32r
AF = mybir.ActivationFunctionType
ALU = mybir.AluOpType

N = 8192
D = 512
NCORES = 8
MY_N = N // NCORES          # 1024 query rows per core
NJT = N // 128              # 64 j-tiles
NIH = MY_N // 512           # 2 i-halves
CLAMP = -80.0

_NC_CACHE = None


def _build_nc():
    nc = bacc.Bacc(None, target_bir_lowering=False)

    xt = nc.dram_tensor("xt", [D, N], F32R, kind="ExternalInput")        # X^T
    xtm = nc.dram_tensor("xtm", [D, MY_N], F32R, kind="ExternalInput")   # X^T slice
    x = nc.dram_tensor("x", [N, D], F32R, kind="ExternalInput")          # X
    wq = nc.dram_tensor("wq", [D, D], F32R, kind="ExternalInput")
    wk = nc.dram_tensor("wk", [D, D], F32R, kind="ExternalInput")
    o = nc.dram_tensor("o", [MY_N, D], F32, kind="ExternalOutput")

    kt_dram = nc.dram_tensor("kt_scratch", [D, N], F32R, kind="Internal")

    with tile.TileContext(nc) as tc:
        with (
            tc.tile_pool(name="pool", bufs=1) as pool,          # persistent
            tc.tile_pool(name="stream", bufs=2) as stream,      # xt blocks / kt blocks
            tc.tile_pool(name="wpool", bufs=1) as wpool,        # wq then wk
            tc.tile_pool(name="big", bufs=1) as big,            # xtm then S-region
            tc.tile_pool(name="ktw", bufs=2) as ktwp,           # K^T write staging
            tc.tile_pool(name="xs", bufs=3) as xsp,             # X tiles (B3)
            tc.tile_pool(name="work", bufs=2) as work,          # d / p / o_sb
            tc.tile_pool(name="ps_qk", bufs=2, space="PSUM") as ps_qk,
            tc.tile_pool(name="ps_o", bufs=1, space="PSUM") as ps_o,
            tc.tile_pool(name="ps_sum", bufs=1, space="PSUM") as ps_sum,
            tc.tile_pool(name="ps_tmp", bufs=1, space="PSUM") as ps_tmp,
        ):
            # ---- constants ----
            ident = pool.tile([128, 128], F32)
            make_identity(nc, ident[:])
            ones_f32 = pool.tile([128, 2], F32)
            nc.vector.memset(ones_f32[:], 1.0)
            ones_col = pool.tile([128, 1], F32R)   # lhsT for column sums
            nc.vector.tensor_copy(ones_col[:], ones_f32[:, 0:1])
            ones_row_f32 = pool.tile([1, 128], F32)
            nc.vector.memset(ones_row_f32[:], 1.0)
            ones_row = pool.tile([1, 128], F32R)   # lhsT for broadcast
            nc.vector.tensor_copy(ones_row[:], ones_row_f32[:])
            one_one = pool.tile([1, 1], F32)
            nc.vector.memset(one_one[:], 1.0)

            qt_sb = pool.tile([128, 4, MY_N], F32R)  # Q^T resident

            # ---- Phase P1: Q^T = (Wq^T X_mine^T) / sqrt(D) ----
            # lhsT = Wq[e-chunk, d-chunk*128 cols]; rhs = X^T_mine[e-chunk, :]
            wq_sb = wpool.tile([128, 4, D], F32R, tag="w")
            nc.sync.dma_start(wq_sb[:], wq[:].rearrange("(c p) d -> p c d", p=128))
            xtm_sb = big.tile([128, 4, MY_N], F32R, tag="big")
            nc.sync.dma_start(xtm_sb[:], xtm[:].rearrange("(c p) i -> p c i", p=128))
            scale = 1.0 / float(np.sqrt(D))
            for dch in range(4):
                for ih in range(NIH):
                    q_ps = ps_qk.tile([128, 512], F32, tag="qk")
                    for e in range(4):
                        nc.tensor.matmul(
                            q_ps[:],
                            wq_sb[:, e, ts(dch, 128)],
                            xtm_sb[:, e, ts(ih, 512)],
                            start=(e == 0),
                            stop=(e == 3),
                        )
                    nc.scalar.activation(
                        qt_sb[:, dch, ts(ih, 512)], q_ps[:], AF.Copy,
                        bias=0.0, scale=scale,
                    )

            # ---- Phase P2: K^T = Wk^T X^T -> DRAM ----
            wk_sb = wpool.tile([128, 4, D], F32R, tag="w")
            nc.sync.dma_start(wk_sb[:], wk[:].rearrange("(c p) d -> p c d", p=128))
            for jb in range(N // 512):  # 16 j-blocks of 512
                xt_blk = stream.tile([128, 4, 512], F32R, tag="stream")
                nc.sync.dma_start(
                    xt_blk[:],
                    xt[:, ts(jb, 512)].rearrange("(c p) j -> p c j", p=128),
                )
                for dch in range(4):
                    k_ps = ps_qk.tile([128, 512], F32, tag="qk")
                    for e in range(4):
                        nc.tensor.matmul(
                            k_ps[:],
                            wk_sb[:, e, ts(dch, 128)],
                            xt_blk[:, e, :],
                            start=(e == 0),
                            stop=(e == 3),
                        )
                    kt_stage = ktwp.tile([128, 512], F32R)
                    nc.scalar.copy(kt_stage[:], k_ps[:])
                    nc.sync.dma_start(
                        kt_dram[ts(dch, 128), ts(jb, 512)], kt_stage[:]
                    )

            # ---- Phase B: per i-half ----
            for h in range(NIH):
                st = big.tile([128, NJT, 512], F32, tag="big")  # S^T region
                mx = pool.tile([128, 512], F32, tag="mx")

                # B1: QK + running max
                for jt in range(NJT):
                    if jt % 4 == 0:
                        kt_blk = stream.tile([128, 4, 512], F32R, tag="stream")
                        nc.sync.dma_start(
                            kt_blk[:],
                            kt_dram[:, ts(jt // 4, 512)].rearrange(
                                "(c p) j -> p c j", p=128
                            ),
                        )
                    s_ps = ps_qk.tile([128, 512], F32, tag="qk")
                    for dch in range(4):
                        nc.tensor.matmul(
                            s_ps[:],
                            kt_blk[:, dch, ts(jt % 4, 128)],
                            qt_sb[:, dch, ts(h, 512)],
                            start=(dch == 0),
                            stop=(dch == 3),
                        )
                    nc.scalar.copy(st[:, jt, :], s_ps[:])
                    if jt == 0:
                        nc.vector.tensor_copy(mx[:], s_ps[:])
                    else:
                        nc.vector.tensor_tensor(mx[:], mx[:], s_ps[:], op=ALU.max)

                # B2: finalize max -> broadcast tile b_sb
                mcol = pool.tile([128, 4], F32, tag="mcol")
                for c in range(4):
                    mt_ps = ps_tmp.tile([128, 128], F32, tag="tmp")
                    nc.tensor.transpose(mt_ps[:], mx[:, ts(c, 128)], ident[:])
                    nc.vector.reduce_max(
                        mcol[:, c : c + 1], mt_ps[:], axis=mybir.AxisListType.X
                    )
                mrow_ps = ps_tmp.tile([1, 512], F32, tag="tmp")
                for c in range(4):
                    nc.tensor.transpose(
                        mrow_ps[:, ts(c, 128)], mcol[:, c : c + 1], ident[:]
                    )
                mrow = pool.tile([1, 512], F32R, tag="mrow")
                nc.scalar.copy(mrow[:], mrow_ps[:])
                b_ps = ps_tmp.tile([128, 512], F32, tag="tmp")
                nc.tensor.matmul(b_ps[:], ones_row[:], mrow[:], start=True, stop=True)
                b_sb = pool.tile([128, 512], F32, tag="bsb")
                nc.scalar.copy(b_sb[:], b_ps[:])

                # B3: exp + attention-weighted accumulation
                o_ps = ps_o.tile([128, 4, 512], F32, tag="o")
                sum_ps = ps_sum.tile([1, 512], F32, tag="sum")
                for jt in range(NJT):
                    x_t = xsp.tile([128, 512], F32R, tag="x")
                    nc.sync.dma_start(x_t[:], x[ts(jt, 128), :])
                    d_t = work.tile([128, 512], F32, tag="d")
                    nc.vector.tensor_tensor(
                        d_t[:], st[:, jt, :], b_sb[:], op=ALU.subtract
                    )
                    nc.vector.tensor_scalar_max(d_t[:], d_t[:], CLAMP)
                    p_t = work.tile([128, 512], F32R, tag="p")
                    nc.scalar.activation(p_t[:], d_t[:], AF.Exp)
                    for c in range(4):
                        nc.tensor.matmul(
                            o_ps[:, c, :],
                            p_t[:, ts(c, 128)],
                            x_t[:],
                            start=(jt == 0),
                            stop=(jt == NJT - 1),
                        )
                    nc.tensor.matmul(
                        sum_ps[:],
                        ones_col[:],
                        p_t[:],
                        start=(jt == 0),
                        stop=(jt == NJT - 1),
                    )

                # B4: normalize + drain
                srow = pool.tile([1, 512], F32, tag="srow")
                nc.scalar.copy(srow[:], sum_ps[:])
                rec_row = pool.tile([1, 512], F32, tag="rec")
                nc.vector.reciprocal(rec_row[:], srow[:])
                rcol = pool.tile([128, 4], F32, tag="rcol")
                for c in range(4):
                    rc_ps = ps_tmp.tile([128, 128], F32, tag="tmp")
                    nc.tensor.transpose(
                        rc_ps[:, 0:1], rec_row[:, ts(c, 128)], one_one[:]
                    )
                    nc.vector.tensor_copy(rcol[:, c : c + 1], rc_ps[:, 0:1])
                for c in range(4):
                    o_sb = work.tile([128, 512], F32, tag="osb")
                    nc.scalar.activation(
                        o_sb[:], o_ps[:, c, :], AF.Copy,
                        bias=0.0, scale=rcol[:, c : c + 1],
                    )
                    nc.sync.dma_start(o[ts(h * 4 + c, 128), :], o_sb[:])

    nc.compile()
    return nc


def _get_nc():
    global _NC_CACHE
    if _NC_CACHE is None:
        _NC_CACHE = _build_nc()
    return _NC_CACHE


def kernel(rotation_params, entangle_params, inputs, _trace=False, _trace_kwargs=None):
    X = np.ascontiguousarray(inputs, dtype=np.float32)
    Wq = np.ascontiguousarray(rotation_params, dtype=np.float32)
    Wk = np.ascontiguousarray(entangle_params, dtype=np.float32)
    XT = np.ascontiguousarray(X.T)

    in_maps = []
    for c in range(NCORES):
        in_maps.append(
            {
                "xt": XT,
                "xtm": np.ascontiguousarray(XT[:, c * MY_N : (c + 1) * MY_N]),
                "x": X,
                "wq": Wq,
                "wk": Wk,
            }
        )

    nc = _get_nc()
    kw = {}
    if _trace:
        kw["trace"] = True
        kw.update(_trace_kwargs or {})
    br = run_bass_kernel_spmd(nc, in_maps, core_ids=list(range(NCORES)), **kw)
    out = np.concatenate([r["o"] for r in br.results], axis=0)
    if _trace:
        return out, br
    return out
